# revision 35
# baseline (speedup 1.0000x reference)
"""Trainium2 Bass kernel for a binarized (1w/1a) BasicBlock — fp8 DoubleRow.

    a1 = sign(x);  y1 = BN(conv3x3(a1, binarize(w1))) + x;  x1 = maxout(y1)
    a2 = sign(x1); y2 = BN(conv3x3(a2, binarize(w2))) + x1; out = maxout(y2)

Data-parallel over batch (4 samples/core, 8 cores); exact binary math:
activations are +-1 (fp8e4, exact), weights are sign(+-1) fp8; each conv is
9 DoubleRow matmuls per (chunk, cout-block), contracting all 256 input
channels at once over contiguous padded-row runs (pad columns land in
unused psum columns).  conv_true = alpha_a*alpha[o]*(BB + q[o]*S1) with
q = beta/alpha; S1 (3x3 box of the channel sum) comes from 3 more DoubleRow
ones-matmuls (folding the kh taps) + 2 shifted adds.  The per-channel scale
folds into BN exactly by scaling BN_EPS per channel.

Batch-stat BN: per-core (sum, sumsq) per channel, one in-place AllReduce per
output-channel block.  Block 0's AllReduce is issued at the conv midpoint so
its mesh latency hides behind block 1's matmuls; block 1's hides behind
block 0's BN-apply.  maxout(t) = A*t + B*|t| with A=(p+n)/2, B=(p-n)/2:
|t| from one Abs activation (bias=shift), out from one DVE STT; the residual
add is folded into the scale STT (t' = scale*cv + x, bias terms carried by
the activations).
"""

import numpy as np
import ml_dtypes

import concourse.bass as bass
import concourse.bacc as bacc
import concourse.mybir as mybir
import concourse.tile as tile

N_CORES = 8
B, C, H, W = 32, 256, 28, 28
BPC = B // N_CORES            # samples per core
NBLK = 2                      # channel blocks of 128
HPAD, WPAD = 30, 30           # padded image in SBUF
PIX = H * W                   # 784
PPIX = HPAD * WPAD            # 900
NCHUNK = 2 * BPC              # 8 chunks of (sample, half-image)
HHALF = H // 2                # 14
CHUNK = HHALF * W             # 392 dense output elems per chunk
RUN = HHALF * WPAD            # 420: rhs run length / psum width per chunk
BN_EPS = 1e-5
NPRM = 24
GUARD = 32                    # fp8 guard elems around merged activation tile
PLANE = BPC * PPIX            # 3600 elems per channel-block plane
SPAN = 4 * CHUNK              # 1568-wide apply spans (2 samples)
NTOT = float(N_CORES * NCHUNK * CHUNK)   # 25088 elems per channel globally
F32 = mybir.dt.float32
FP8 = mybir.dt.float8e4
AF = mybir.ActivationFunctionType
ALU = mybir.AluOpType
DR = mybir.MatmulPerfMode.DoubleRow


def _evac(nc, sc, ps, s1, stat, cv, prm, pcol, ci, oblk):
    """z = q[o]*S1 + BB from PSUM (strided: skip pad cols).  Stats come for
    free: the STT accumulates sum(z) on DVE; a Square pass on the otherwise
    idle ScalarE accumulates sum(z^2).  Layout of stat: per block 16 cols,
    [sum(8) | sumsq(8)], so one DMA ships the block's raw accumulators."""
    psv = ps[:].rearrange("p (h w) -> p h w", h=HHALF)[:, :, 1:1 + W]
    s1v = s1[:].rearrange("p (h w) -> p h w", h=H)[
        :, (ci % 2) * HHALF:(ci % 2) * HHALF + HHALF, :]
    cvc = cv[oblk][:, ci * CHUNK:(ci + 1) * CHUNK]
    c0 = oblk * 2 * NCHUNK
    nc.vector.scalar_tensor_tensor(
        cvc.rearrange("p (h w) -> p h w", h=HHALF), s1v,
        prm[:, pcol['q'] + oblk:pcol['q'] + oblk + 1], psv,
        op0=ALU.mult, op1=ALU.add,
        accum_out=stat[:, c0 + ci:c0 + ci + 1])
    sqj = sc.tile([128, CHUNK], F32, tag="sqj", name="sqj", bufs=2)
    nc.scalar.activation(
        sqj[:], cvc, AF.Square,
        accum_out=stat[:, c0 + NCHUNK + ci:c0 + NCHUNK + ci + 1])


def _stats_pre(nc, rnd, oblk, stat, s_d, g_d):
    """Raw block accumulators -> DRAM (issued from the ScalarE queue right
    after the block's last Square, no cross-engine reduce hop) -> AllGather.
    Only the trigger lives on gpsimd (straight-line collective order)."""
    c0 = oblk * 2 * NCHUNK
    nc.scalar.dma_start(s_d[:], stat[:, c0:c0 + 2 * NCHUNK])
    nc.gpsimd.collective_compute(
        "AllGather", ALU.bypass,
        replica_groups=[list(range(N_CORES))],
        ins=[s_d.opt()], outs=[g_d.opt()])


def _stats_post(nc, sbuf, rnd, oblk, g_d, prm, pcol):
    """Split path: read back this block's AllGather and finish BN."""
    nrc = N_CORES * NCHUNK
    gst = sbuf.tile([128, 2 * nrc], F32, name=f"gst{rnd}_{oblk}")
    nc.sync.dma_start(
        gst[:].rearrange("p (q r c) -> p q r c", q=2, r=N_CORES),
        g_d[:].rearrange("r p (q c) -> p q r c", q=2))
    return _bn_math(nc, sbuf, rnd, oblk, gst[:, 0:nrc], gst[:, nrc:2 * nrc],
                    prm, pcol)


def _bn_math(nc, sbuf, rnd, oblk, gsum, gsq, prm, pcol):
    """Reduce gathered partials -> scale/shift/A*shift for this block."""
    w = sbuf.tile([128, 10], F32, name=f"stw{rnd}_{oblk}")
    mean, qn = w[:, 2:3], w[:, 3:4]
    m2, var, inv = w[:, 4:5], w[:, 5:6], w[:, 6:7]
    scale, shift, asx = w[:, 7:8], w[:, 8:9], w[:, 9:10]
    nc.vector.reduce_sum(w[:, 0:1], gsum, axis=mybir.AxisListType.X)
    nc.vector.reduce_sum(w[:, 1:2], gsq, axis=mybir.AxisListType.X)
    nc.vector.tensor_scalar_mul(mean, w[:, 0:1], 1.0 / NTOT)
    nc.vector.tensor_scalar_mul(qn, w[:, 1:2], 1.0 / NTOT)
    nc.vector.tensor_mul(m2, mean, mean)
    nc.vector.scalar_tensor_tensor(var, m2, -1.0, qn,
                                   op0=ALU.mult, op1=ALU.add)
    nc.scalar.activation(m2, var, AF.Sqrt,
                         bias=prm[:, pcol['eps'] + oblk:pcol['eps'] + oblk + 1],
                         scale=1.0)                   # reuse m2 as sd
    nc.vector.reciprocal(inv, m2)
    nc.vector.tensor_mul(scale, inv,
                         prm[:, pcol['g'] + oblk:pcol['g'] + oblk + 1])
    nc.vector.tensor_mul(qn, mean, scale)     # reuse qn as tmp
    nc.vector.tensor_sub(shift,
                         prm[:, pcol['b'] + oblk:pcol['b'] + oblk + 1], qn)
    nc.vector.tensor_mul(asx, shift,
                         prm[:, pcol['hs'] + oblk:pcol['hs'] + oblk + 1])
    return scale, shift, asx


def _apply_a(nc, sc, oblk, b, cv, xres, sss, prm, pcol, a2v):
    """Phase A — the only part conv-2 waits on: t' = scale*cv + x (DVE),
    sign(t'+shift) -> next conv's padded fp8 input (ScalarE).  Returns the
    t' tile for phase B."""
    scale, shift, asx = sss
    cvs = cv[oblk][:, b * PIX:(b + 1) * PIX]
    xrs = xres[oblk][:, b * PIX:(b + 1) * PIX]
    tp = sc.tile([128, PIX], F32, tag="tp", name="tp", bufs=8)
    nc.vector.scalar_tensor_tensor(tp[:], cvs, scale, xrs,
                                   op0=ALU.mult, op1=ALU.add)
    if a2v is not None:
        sg = a2v[:, b, oblk, 1:1 + H, 1:1 + W]
        nc.scalar.activation(
            sg, tp[:].rearrange("p (h w) -> p h w", h=H),
            AF.Sign, bias=shift, scale=1.0)
    return tp


def _apply_b(nc, sc, oblk, b, tp, xres, sss, prm, pcol, ov=None):
    """Phase B — maxout finish, deferred off conv-2's critical path:
    |t| = Abs(t'+shift) (ScalarE), v = B*|t| + A*shift (GpSimd — on DVE
    it serializes with tp/out and costs ~15us in the round-2 tail),
    out = A*t' + v (DVE; in-place into xres, or staged + DMA)."""
    scale, shift, asx = sss
    xrs = xres[oblk][:, b * PIX:(b + 1) * PIX]
    u1 = sc.tile([128, PIX], F32, tag="u1", name="u1", bufs=3)
    nc.scalar.activation(u1[:], tp[:], AF.Abs, bias=shift, scale=1.0)
    v = sc.tile([128, PIX], F32, tag="v", name="v", bufs=3)
    nc.gpsimd.tensor_scalar(v[:], u1[:],
                            prm[:, pcol['hp'] + oblk:pcol['hp'] + oblk + 1],
                            asx, op0=ALU.mult, op1=ALU.add)
    if ov is None:
        nc.vector.scalar_tensor_tensor(
            xrs, tp[:], prm[:, pcol['hs'] + oblk:pcol['hs'] + oblk + 1],
            v[:], op0=ALU.mult, op1=ALU.add)
    else:
        och = sc.tile([128, PIX], F32, tag="och", name="och", bufs=3)
        nc.vector.scalar_tensor_tensor(
            och[:], tp[:], prm[:, pcol['hs'] + oblk:pcol['hs'] + oblk + 1],
            v[:], op0=ALU.mult, op1=ALU.add)
        nc.sync.dma_start(ov[oblk * 128:oblk * 128 + 128, b], och[:])


def _apply(nc, sc, oblk, b, cv, xres, sss, prm, pcol, a2v=None, ov=None):
    tp = _apply_a(nc, sc, oblk, b, cv, xres, sss, prm, pcol, a2v)
    _apply_b(nc, sc, oblk, b, tp, xres, sss, prm, pcol, ov=ov)


def _round(nc, tc, pools, rnd, apad, wsb, xres, cv, prm, pcol, onesb,
           out_d=None, a2pad=None, split_stats=True, defer_finish=False,
           finish_prev=None):
    """Full conv+BN+residual+maxout round.

    split_stats=True: one AllGather per output-channel block (block 0's
    mesh overlaps block 1's matmuls) — right when the CC core is warm and
    free (round 2).  split_stats=False: a single AllGather for both blocks
    — right for round 1, where the CC core ignores collectives until ~60us
    and is strictly serial, so two meshes would just queue behind the
    warmup and delay the apply."""
    sbuf, psum, sc, dram = pools
    stat = sbuf.tile([128, 2 * NCHUNK * NBLK], F32, name=f"stat{rnd}")
    if split_stats:
        s_ds = [dram.tile([128, 2 * NCHUNK], F32, name=f"sd{rnd}_{o}")
                for o in range(NBLK)]
        g_ds = [dram.tile([N_CORES, 128, 2 * NCHUNK], F32, name=f"gd{rnd}_{o}")
                for o in range(NBLK)]
    else:
        s_dm = dram.tile([128, 2 * NCHUNK * NBLK], F32, name=f"sdm{rnd}")
        g_dm = dram.tile([N_CORES, 128, 2 * NCHUNK * NBLK], F32,
                         name=f"gdm{rnd}")
    # sample-major planes: (b i h w), DoubleRow i-stride = PPIX, so each
    # matmul's rhs bounding range stays inside its own sample's planes
    t420 = apad[:, 0:2 * PLANE].rearrange(
        "p (b i n) -> p b i n", b=BPC, i=2)[:, 0, :, 0:RUN]

    def rhs_ap(off):
        return bass.AP(t420.tensor, GUARD + off, t420.ap)

    wv = wsb[:].rearrange("p (k i o) -> p k i o", k=9, i=NBLK)
    ones3 = onesb[:].rearrange("p (i o) -> p i o", i=2)

    # ---- S1 pipeline for all samples first (keeps the main MM stream
    # free of evac dependencies), then main matmuls with immediate evac ----
    s1s = {}
    for b in range(BPC):
        hs = sc.tile([128, 2 * RUN], F32, tag="hs", name="hs", bufs=2)
        for half in range(2):
            h0 = half * HHALF
            ps2 = psum.tile([128, RUN], F32, tag="ps2", name=f"ps2_{rnd}",
                            bufs=4)
            for kh in range(3):
                nc.tensor.matmul(ps2[:], ones3,
                                 rhs_ap(b * 2 * PPIX + (h0 + kh) * WPAD),
                                 start=(kh == 0), stop=(kh == 2), perf_mode=DR)
            nc.scalar.copy(hs[:, half * RUN:half * RUN + RUN], ps2[:])
        # W-direction 3-tap over the whole sample (28 rows x 30)
        hsv = hs[:].rearrange("p (h w) -> p h w", h=H)
        w3 = sc.tile([128, H * W], F32, tag="w3", name="w3", bufs=2)
        w3v = w3[:].rearrange("p (h w) -> p h w", h=H)
        nc.gpsimd.tensor_add(w3v, hsv[:, :, 0:W], hsv[:, :, 1:1 + W])
        s1 = sc.tile([128, H * W], F32, tag="s1", name="s1", bufs=BPC)
        s1v = s1[:].rearrange("p (h w) -> p h w", h=H)
        nc.vector.tensor_add(s1v, w3v, hsv[:, :, 2:2 + W])
        s1s[b] = s1

    if a2pad is not None:
        a2v = a2pad[:, GUARD:GUARD + 2 * PLANE].rearrange(
            "p (b i h w) -> p b i h w", b=BPC, i=2, h=HPAD, w=WPAD)
    else:
        a2v = None
    if out_d is not None:
        ov = out_d[:].rearrange("b c h w -> c b (h w)")
    else:
        ov = None

    # block-major main loop: block 0's AllReduce is triggered while block 1
    # is still on the PE array; both applies trail their collective.
    for oblk in range(NBLK):
        for ci in range(NCHUNK):
            b, h0 = ci // 2, (ci % 2) * HHALF
            ps = psum.tile([128, RUN], F32, tag="ps", name=f"ps{rnd}",
                           bufs=4)
            for k9 in range(9):
                kh, kw = k9 // 3, k9 % 3
                nc.tensor.matmul(
                    ps[:], wv[:, k9, :, oblk * 128:(oblk + 1) * 128],
                    rhs_ap(b * 2 * PPIX + (h0 + kh) * WPAD + kw - 1),
                    start=(k9 == 0), stop=(k9 == 8), perf_mode=DR)
            _evac(nc, sc, ps, s1s[b], stat, cv, prm, pcol, ci, oblk)
        if split_stats:
            _stats_pre(nc, rnd, oblk, stat, s_ds[oblk], g_ds[oblk])
        # the previous round's deferred maxout-finish ops slot in here:
        # after this round's block-0 collective is on its way, before the
        # block-1 stats chain — their engines are otherwise waiting
        if oblk == 0 and finish_prev is not None:
            finish_prev()
            finish_prev = None

    if split_stats:
        fins = []
        for oblk in range(NBLK):
            sss = _stats_post(nc, sbuf, rnd, oblk, g_ds[oblk], prm, pcol)
            for b in range(BPC):
                if defer_finish:
                    tp = _apply_a(nc, sc, oblk, b, cv, xres, sss, prm,
                                  pcol, a2v)
                    fins.append((oblk, b, tp, sss))
                else:
                    _apply(nc, sc, oblk, b, cv, xres, sss, prm, pcol,
                           a2v=a2v, ov=ov)
        if defer_finish:
            def finish():
                for oblk, b, tp, sss in fins:
                    _apply_b(nc, sc, oblk, b, tp, xres, sss, prm, pcol,
                             ov=ov)
            return finish
    else:
        nc.scalar.dma_start(s_dm[:], stat[:])
        nc.gpsimd.collective_compute(
            "AllGather", ALU.bypass,
            replica_groups=[list(range(N_CORES))],
            ins=[s_dm.opt()], outs=[g_dm.opt()])
        nrc = N_CORES * NCHUNK
        gst = sbuf.tile([128, NBLK * 2 * nrc], F32, name=f"gstm{rnd}")
        nc.sync.dma_start(
            gst[:].rearrange("p (o q r c) -> p o q r c", o=NBLK, q=2,
                             r=N_CORES),
            g_dm[:].rearrange("r p (o q c) -> p o q r c", o=NBLK, q=2))
        ssss = [_bn_math(nc, sbuf, rnd, o,
                         gst[:, o * 2 * nrc:o * 2 * nrc + nrc],
                         gst[:, o * 2 * nrc + nrc:(o + 1) * 2 * nrc],
                         prm, pcol)
                for o in range(NBLK)]
        # sample-major so a2 for sample b completes after two applies and
        # round 2's S1 pipeline starts immediately
        for b in range(BPC):
            for oblk in range(NBLK):
                _apply(nc, sc, oblk, b, cv, xres, ssss[oblk], prm, pcol,
                       a2v=a2v, ov=ov)


def build():
    nc = bacc.Bacc("TRN2", target_bir_lowering=False, debug=False,
                   enable_asserts=True, num_devices=N_CORES)
    x_d = nc.dram_tensor("x", [BPC, C, H, W], F32, kind="ExternalInput")
    w1_d = nc.dram_tensor("w1t", [9, NBLK, 128, 256], FP8,
                          kind="ExternalInput")
    w2_d = nc.dram_tensor("w2t", [9, NBLK, 128, 256], FP8,
                          kind="ExternalInput")
    prm_d = nc.dram_tensor("prm", [128, NPRM], F32, kind="ExternalInput")
    out_d = nc.dram_tensor("out", [BPC, C, H, W], F32, kind="ExternalOutput")

    with tile.TileContext(nc) as tc:
        with (
            tc.tile_pool(name="sbuf", bufs=1) as sbuf,
            tc.tile_pool(name="psum", bufs=6, space="PSUM") as psum,
            tc.tile_pool(name="sc", bufs=2) as sc,
            tc.tile_pool(name="dram", bufs=1, space="DRAM") as dram,
        ):
            pools = (sbuf, psum, sc, dram)
            # Warmup AllGather: the CC cores ignore user collectives until
            # ~50us (NRT init) and the first mesh pays ~11us setup + cold
            # data path; this junk collective eats those costs and syncs
            # the 8 cores so the real stats meshes run warm (~5us).
            wu_i = dram.tile([1, 16], F32, name="wu_i")
            wu_o = dram.tile([N_CORES, 16], F32, name="wu_o")
            wu_s = sbuf.tile([1, 16], F32, name="wu_s")
            nc.vector.memset(wu_s[:], 0.0)
            nc.scalar.dma_start(wu_i[:], wu_s[:])
            nc.gpsimd.collective_compute(
                "AllGather", ALU.bypass,
                replica_groups=[list(range(N_CORES))],
                ins=[wu_i.opt()], outs=[wu_o.opt()])
            w1sb = sbuf.tile([128, 9 * NBLK * 256], FP8, name="w1sb")
            w2sb = sbuf.tile([128, 9 * NBLK * 256], FP8, name="w2sb")
            prm = sbuf.tile([128, NPRM], F32, name="prm")
            onesb = sbuf.tile([128, 256], FP8, name="onesb")
            nc.vector.memset(onesb[:], 1.0)
            xres = [sbuf.tile([128, BPC * PIX], F32, name=f"xres{i}")
                    for i in range(NBLK)]
            a1p = sbuf.tile([128, GUARD + 2 * PLANE + GUARD], FP8, name="a1p")
            a2p = sbuf.tile([128, GUARD + 2 * PLANE + GUARD], FP8, name="a2p")
            cv = [sbuf.tile([128, BPC * PIX], F32, name=f"cv{i}")
                  for i in range(NBLK)]

            # zero the padded activation tiles (pads must read 0); split the
            # a1 memset across DVE+GpSimd so sample-0 signing starts early
            hlen = (GUARD + 2 * PLANE + GUARD) // 2
            nc.vector.memset(a1p[:, 0:hlen].bitcast(mybir.dt.uint32), 0)
            nc.gpsimd.memset(a1p[:, hlen:].bitcast(mybir.dt.uint32), 0)
            nc.sync.dma_start(prm[:], prm_d[:])
            xv = x_d[:].rearrange("b c h w -> c b (h w)")
            for b in range(BPC):
                eng = nc.sync if b < 2 else nc.scalar
                for i in range(NBLK):
                    eng.dma_start(
                        xres[i][:, b * PIX:(b + 1) * PIX],
                        xv[i * 128:(i + 1) * 128, b])
            # w1 split across three DMA queues so no single transfer
            # gates the first matmul
            w1v = w1sb[:].rearrange("p (k i o) -> p k i o", k=9, i=NBLK)
            w1dv = w1_d[:].rearrange("k i p o -> p k i o")
            nc.scalar.dma_start(w1v[:, 0:3], w1dv[:, 0:3])
            nc.gpsimd.dma_start(w1v[:, 3:6], w1dv[:, 3:6])
            nc.sync.dma_start(w1v[:, 6:9], w1dv[:, 6:9])
            nc.gpsimd.memset(a2p[:].bitcast(mybir.dt.uint32), 0)
            nc.sync.dma_start(
                w2sb[:].rearrange("p (k i o) -> p k i o", k=9, i=NBLK),
                w2_d[:].rearrange("k i p o -> p k i o"))
            # a1 = sign(x) (+-1) into padded interior of merged fp8 tile
            a1v = a1p[:, GUARD:GUARD + 2 * PLANE].rearrange(
                "p (b i h w) -> p b i h w", b=BPC, i=2, h=HPAD, w=WPAD)
            xrvs = [xres[i][:].rearrange("p (b h w) -> p b h w", b=BPC, h=H)
                    for i in range(NBLK)]
            for b in range(BPC):
                for i in range(NBLK):
                    nc.scalar.activation(a1v[:, b, i, 1:1 + H, 1:1 + W],
                                         xrvs[i][:, b], AF.Sign)

            pcol1 = {'g': 0, 'b': 2, 'hp': 4, 'hs': 6, 'eps': 16, 'q': 20}
            pcol2 = {'g': 8, 'b': 10, 'hp': 12, 'hs': 14, 'eps': 18, 'q': 22}
            f1 = _round(nc, tc, pools, 1, a1p, w1sb, xres, cv, prm, pcol1,
                        onesb, a2pad=a2p, split_stats=True,
                        defer_finish=True)
            _round(nc, tc, pools, 2, a2p, w2sb, xres, cv, prm, pcol2, onesb,
                   out_d=out_d, split_stats=True, finish_prev=f1)

    nc.compile()
    return nc


def _prep_weight(w):
    """(O,I,3,3) fp32 -> sign lhsT (9, iblk, 128, 256) fp8 (+-1, exact),
    plus per-output-channel alpha, beta (float64)."""
    w = w.astype(np.float64)
    beta = w.mean(axis=(1, 2, 3))
    alpha = np.sqrt(((w - beta[:, None, None, None]) ** 2)
                    .mean(axis=(1, 2, 3)))
    s = np.sign(w - beta[:, None, None, None]).astype(np.float32)
    wt = s.transpose(2, 3, 1, 0).reshape(9, C, C)   # (k9, i, o)
    wt = wt.reshape(9, NBLK, 128, C)                # (k9, iblk, i, o)
    return wt.astype(ml_dtypes.float8_e4m3), alpha, beta


def make_in_maps(inputs):
    x = np.asarray(inputs['x'], np.float32)
    aa1 = float(np.asarray(inputs['alpha_a1']).reshape(-1)[0])
    aa2 = float(np.asarray(inputs['alpha_a2']).reshape(-1)[0])
    w1t, al1, be1 = _prep_weight(np.asarray(inputs['w1'], np.float32))
    w2t, al2, be2 = _prep_weight(np.asarray(inputs['w2'], np.float32))
    prm = np.zeros((128, NPRM), np.float32)
    f1 = 1.0 / (aa1 * al1)      # z scale relative to the true conv output
    f2 = 1.0 / (aa2 * al2)
    p1 = np.asarray(inputs['pos1'], np.float64)
    n1 = np.asarray(inputs['neg1'], np.float64)
    p2 = np.asarray(inputs['pos2'], np.float64)
    n2 = np.asarray(inputs['neg2'], np.float64)
    cols = ((0, np.asarray(inputs['g1'], np.float64)),
            (2, np.asarray(inputs['b1'], np.float64)),
            (4, (p1 - n1) / 2),
            (6, (p1 + n1) / 2),
            (8, np.asarray(inputs['g2'], np.float64)),
            (10, np.asarray(inputs['b2'], np.float64)),
            (12, (p2 - n2) / 2),
            (14, (p2 + n2) / 2),
            (16, BN_EPS * f1 * f1),
            (18, BN_EPS * f2 * f2),
            (20, be1 / al1),
            (22, be2 / al2))
    for base, arr in cols:
        prm[:, base] = arr[:128]
        prm[:, base + 1] = arr[128:]
    in_maps = []
    for c in range(N_CORES):
        in_maps.append({
            'x': np.ascontiguousarray(x[c * BPC:(c + 1) * BPC]),
            'w1t': w1t, 'w2t': w2t, 'prm': prm,
        })
    return in_maps


_CACHE = {}


def kernel(**inputs):
    in_maps = make_in_maps(inputs)
    if 'run' not in _CACHE:
        nc = build()
        _CACHE['nc'] = nc
        _CACHE['run'] = _make_runner(nc)
    outs = _CACHE['run'](in_maps)
    return np.concatenate([outs[c] for c in range(N_CORES)], axis=0)


def _make_runner(nc):
    """Build a cached PJRT executable (same path run_bass_kernel_spmd takes
    under axon, via bass2jax) so repeat calls don't re-trace."""
    import jax
    import jax.numpy as jnp
    from jax.sharding import Mesh, PartitionSpec
    from jax.experimental.shard_map import shard_map
    from concourse import bass2jax

    bass2jax.install_neuronx_cc_hook()
    partition_name = (nc.partition_id_tensor.name
                      if nc.partition_id_tensor else None)
    in_names = []
    out_names = []
    out_avals = []
    for alloc in nc.m.functions[0].allocations:
        if not isinstance(alloc, mybir.MemoryLocationSet):
            continue
        name = alloc.memorylocations[0].name
        if alloc.kind == "ExternalInput":
            if name != partition_name:
                in_names.append(name)
        elif alloc.kind == "ExternalOutput":
            shape = tuple(alloc.tensor_shape)
            dtype = mybir.dt.np(alloc.dtype)
            out_names.append(name)
            out_avals.append(jax.core.ShapedArray(shape, dtype))
    n_params = len(in_names)
    all_names = in_names + out_names
    if partition_name is not None:
        all_names = all_names + [partition_name]

    def _body(*args):
        operands = list(args)
        if partition_name is not None:
            operands.append(bass2jax.partition_id_tensor())
        outs = bass2jax._bass_exec_p.bind(
            *operands,
            out_avals=tuple(out_avals),
            in_names=tuple(all_names),
            out_names=tuple(out_names),
            lowering_input_output_aliases=(),
            sim_require_finite=True,
            sim_require_nnan=True,
            nc=nc,
        )
        return tuple(outs)

    devices = jax.devices()[:N_CORES]
    mesh = Mesh(np.asarray(devices), ("core",))
    n_outs = len(out_names)
    sharded = jax.jit(
        shard_map(_body, mesh=mesh,
                  in_specs=(PartitionSpec("core"),) * (n_params + n_outs),
                  out_specs=(PartitionSpec("core"),) * n_outs,
                  check_rep=False),
        donate_argnums=tuple(range(n_params, n_params + n_outs)),
        keep_unused=True,
    )
    sharded_nodonate = jax.jit(
        shard_map(_body, mesh=mesh,
                  in_specs=(PartitionSpec("core"),) * (n_params + n_outs),
                  out_specs=(PartitionSpec("core"),) * n_outs,
                  check_rep=False),
        keep_unused=True,
    )

    def run(in_maps):
        concat_in = [
            np.concatenate([np.asarray(in_maps[c][n]) for c in range(N_CORES)],
                           axis=0)
            for n in in_names
        ]
        concat_zeros = [
            np.zeros((N_CORES * a.shape[0], *a.shape[1:]), a.dtype)
            for a in out_avals
        ]
        out_arrs = sharded(*concat_in, *concat_zeros)
        i = out_names.index("out")
        full = np.asarray(out_arrs[i]).reshape(N_CORES, *out_avals[i].shape)
        return [full[c] for c in range(N_CORES)]

    def stage(in_maps):
        """device_put inputs once; return a dispatch closure for timing."""
        from jax.sharding import NamedSharding
        sh = NamedSharding(mesh, PartitionSpec("core"))
        concat_in = [
            jax.device_put(np.concatenate(
                [np.asarray(in_maps[c][n]) for c in range(N_CORES)], axis=0), sh)
            for n in in_names
        ]
        concat_zeros = [
            jax.device_put(
                np.zeros((N_CORES * a.shape[0], *a.shape[1:]), a.dtype), sh)
            for a in out_avals
        ]

        def dispatch():
            return sharded_nodonate(*concat_in, *concat_zeros)

        return dispatch

    run.stage = stage
    return run


# revision 37
# speedup vs baseline: 1.0341x; 1.0341x over previous
"""Trainium2 Bass kernel for a binarized (1w/1a) BasicBlock — fp8 DoubleRow.

    a1 = sign(x);  y1 = BN(conv3x3(a1, binarize(w1))) + x;  x1 = maxout(y1)
    a2 = sign(x1); y2 = BN(conv3x3(a2, binarize(w2))) + x1; out = maxout(y2)

Data-parallel over batch (4 samples/core, 8 cores); exact binary math:
activations are +-1 (fp8e4, exact), weights are sign(+-1) fp8; each conv is
9 DoubleRow matmuls per (chunk, cout-block), contracting all 256 input
channels at once over contiguous padded-row runs (pad columns land in
unused psum columns).  conv_true = alpha_a*alpha[o]*(BB + q[o]*S1) with
q = beta/alpha; S1 (3x3 box of the channel sum) comes from 3 more DoubleRow
ones-matmuls (folding the kh taps) + 2 shifted adds.  The per-channel scale
folds into BN exactly by scaling BN_EPS per channel.

Batch-stat BN: per-core (sum, sumsq) per channel, one in-place AllReduce per
output-channel block.  Block 0's AllReduce is issued at the conv midpoint so
its mesh latency hides behind block 1's matmuls; block 1's hides behind
block 0's BN-apply.  maxout(t) = A*t + B*|t| with A=(p+n)/2, B=(p-n)/2:
|t| from one Abs activation (bias=shift), out from one DVE STT; the residual
add is folded into the scale STT (t' = scale*cv + x, bias terms carried by
the activations).
"""

import numpy as np
import ml_dtypes

import concourse.bass as bass
import concourse.bacc as bacc
import concourse.mybir as mybir
import concourse.tile as tile

N_CORES = 8
B, C, H, W = 32, 256, 28, 28
BPC = B // N_CORES            # samples per core
NBLK = 2                      # channel blocks of 128
HPAD, WPAD = 30, 30           # padded image in SBUF
PIX = H * W                   # 784
PPIX = HPAD * WPAD            # 900
NCHUNK = 2 * BPC              # 8 chunks of (sample, half-image)
HHALF = H // 2                # 14
CHUNK = HHALF * W             # 392 dense output elems per chunk
RUN = HHALF * WPAD            # 420: rhs run length / psum width per chunk
BN_EPS = 1e-5
NPRM = 24
GUARD = 32                    # fp8 guard elems around merged activation tile
PLANE = BPC * PPIX            # 3600 elems per channel-block plane
SPAN = 4 * CHUNK              # 1568-wide apply spans (2 samples)
NTOT = float(N_CORES * NCHUNK * CHUNK)   # 25088 elems per channel globally
F32 = mybir.dt.float32
FP8 = mybir.dt.float8e4
AF = mybir.ActivationFunctionType
ALU = mybir.AluOpType
DR = mybir.MatmulPerfMode.DoubleRow


def _evac(nc, sc, ps, s1, stat, cv, prm, pcol, ci, oblk):
    """z = q[o]*S1 + BB from PSUM (strided: skip pad cols).  Stats come for
    free: the STT accumulates sum(z) on DVE; a Square pass on the otherwise
    idle ScalarE accumulates sum(z^2).  Layout of stat: per block 16 cols,
    [sum(8) | sumsq(8)], so one DMA ships the block's raw accumulators."""
    psv = ps[:].rearrange("p (h w) -> p h w", h=HHALF)[:, :, 1:1 + W]
    s1v = s1[:].rearrange("p (h w) -> p h w", h=H)[
        :, (ci % 2) * HHALF:(ci % 2) * HHALF + HHALF, :]
    cvc = cv[oblk][:, ci * CHUNK:(ci + 1) * CHUNK]
    c0 = oblk * 2 * NCHUNK
    nc.vector.scalar_tensor_tensor(
        cvc.rearrange("p (h w) -> p h w", h=HHALF), s1v,
        prm[:, pcol['q'] + oblk:pcol['q'] + oblk + 1], psv,
        op0=ALU.mult, op1=ALU.add,
        accum_out=stat[:, c0 + ci:c0 + ci + 1])
    sqj = sc.tile([128, CHUNK], F32, tag="sqj", name="sqj", bufs=2)
    nc.scalar.activation(
        sqj[:], cvc, AF.Square,
        accum_out=stat[:, c0 + NCHUNK + ci:c0 + NCHUNK + ci + 1])


def _stats_pre(nc, rnd, oblk, stat, s_d, g_d):
    """Raw block accumulators -> DRAM (issued from the ScalarE queue right
    after the block's last Square, no cross-engine reduce hop) -> AllGather.
    Only the trigger lives on gpsimd (straight-line collective order)."""
    c0 = oblk * 2 * NCHUNK
    nc.scalar.dma_start(s_d[:], stat[:, c0:c0 + 2 * NCHUNK])
    nc.gpsimd.collective_compute(
        "AllGather", ALU.bypass,
        replica_groups=[list(range(N_CORES))],
        ins=[s_d.opt()], outs=[g_d.opt()])


def _stats_post(nc, sbuf, rnd, oblk, g_d, prm, pcol):
    """Split path: read back this block's AllGather and finish BN."""
    nrc = N_CORES * NCHUNK
    gst = sbuf.tile([128, 2 * nrc], F32, name=f"gst{rnd}_{oblk}")
    nc.sync.dma_start(
        gst[:].rearrange("p (q r c) -> p q r c", q=2, r=N_CORES),
        g_d[:].rearrange("r p (q c) -> p q r c", q=2))
    return _bn_math(nc, sbuf, rnd, oblk, gst[:, 0:nrc], gst[:, nrc:2 * nrc],
                    prm, pcol)


def _bn_math(nc, sbuf, rnd, oblk, gsum, gsq, prm, pcol):
    """Reduce gathered partials -> scale/shift/A*shift for this block."""
    w = sbuf.tile([128, 10], F32, name=f"stw{rnd}_{oblk}")
    mean, qn = w[:, 2:3], w[:, 3:4]
    m2, var, inv = w[:, 4:5], w[:, 5:6], w[:, 6:7]
    scale, shift, asx = w[:, 7:8], w[:, 8:9], w[:, 9:10]
    nc.vector.reduce_sum(w[:, 0:1], gsum, axis=mybir.AxisListType.X)
    nc.vector.reduce_sum(w[:, 1:2], gsq, axis=mybir.AxisListType.X)
    nc.vector.tensor_scalar_mul(mean, w[:, 0:1], 1.0 / NTOT)
    nc.vector.tensor_scalar_mul(qn, w[:, 1:2], 1.0 / NTOT)
    nc.vector.tensor_mul(m2, mean, mean)
    nc.vector.scalar_tensor_tensor(var, m2, -1.0, qn,
                                   op0=ALU.mult, op1=ALU.add)
    nc.scalar.activation(m2, var, AF.Sqrt,
                         bias=prm[:, pcol['eps'] + oblk:pcol['eps'] + oblk + 1],
                         scale=1.0)                   # reuse m2 as sd
    nc.vector.reciprocal(inv, m2)
    nc.vector.tensor_mul(scale, inv,
                         prm[:, pcol['g'] + oblk:pcol['g'] + oblk + 1])
    nc.vector.tensor_mul(qn, mean, scale)     # reuse qn as tmp
    nc.vector.tensor_sub(shift,
                         prm[:, pcol['b'] + oblk:pcol['b'] + oblk + 1], qn)
    nc.vector.tensor_mul(asx, shift,
                         prm[:, pcol['hs'] + oblk:pcol['hs'] + oblk + 1])
    return scale, shift, asx


def _apply_a(nc, sc, oblk, b, cv, xres, sss, prm, pcol, a2v):
    """Phase A — the only part conv-2 waits on: t' = scale*cv + x (DVE),
    sign(t'+shift) -> next conv's padded fp8 input (ScalarE).  Returns the
    t' tile for phase B."""
    scale, shift, asx = sss
    cvs = cv[oblk][:, b * PIX:(b + 1) * PIX]
    xrs = xres[oblk][:, b * PIX:(b + 1) * PIX]
    tp = sc.tile([128, PIX], F32, tag="tp", name="tp", bufs=8)
    nc.vector.scalar_tensor_tensor(tp[:], cvs, scale, xrs,
                                   op0=ALU.mult, op1=ALU.add)
    if a2v is not None:
        sg = a2v[:, b, oblk, 1:1 + H, 1:1 + W]
        nc.scalar.activation(
            sg, tp[:].rearrange("p (h w) -> p h w", h=H),
            AF.Sign, bias=shift, scale=1.0)
    return tp


def _apply_b(nc, sc, oblk, b, tp, xres, sss, prm, pcol, ov=None):
    """Phase B — maxout finish, deferred off conv-2's critical path:
    |t| = Abs(t'+shift) (ScalarE), v = B*|t| + A*shift (GpSimd — on DVE
    it serializes with tp/out and costs ~15us in the round-2 tail),
    out = A*t' + v (DVE; in-place into xres, or staged + DMA)."""
    scale, shift, asx = sss
    xrs = xres[oblk][:, b * PIX:(b + 1) * PIX]
    u1 = sc.tile([128, PIX], F32, tag="u1", name="u1", bufs=3)
    nc.scalar.activation(u1[:], tp[:], AF.Abs, bias=shift, scale=1.0)
    v = sc.tile([128, PIX], F32, tag="v", name="v", bufs=3)
    nc.gpsimd.tensor_scalar(v[:], u1[:],
                            prm[:, pcol['hp'] + oblk:pcol['hp'] + oblk + 1],
                            asx, op0=ALU.mult, op1=ALU.add)
    if ov is None:
        nc.vector.scalar_tensor_tensor(
            xrs, tp[:], prm[:, pcol['hs'] + oblk:pcol['hs'] + oblk + 1],
            v[:], op0=ALU.mult, op1=ALU.add)
    else:
        och = sc.tile([128, PIX], F32, tag="och", name="och", bufs=3)
        nc.vector.scalar_tensor_tensor(
            och[:], tp[:], prm[:, pcol['hs'] + oblk:pcol['hs'] + oblk + 1],
            v[:], op0=ALU.mult, op1=ALU.add)
        nc.sync.dma_start(ov[oblk * 128:oblk * 128 + 128, b], och[:])


def _apply(nc, sc, oblk, b, cv, xres, sss, prm, pcol, a2v=None, ov=None):
    tp = _apply_a(nc, sc, oblk, b, cv, xres, sss, prm, pcol, a2v)
    _apply_b(nc, sc, oblk, b, tp, xres, sss, prm, pcol, ov=ov)


def _round(nc, tc, pools, rnd, apad, wsb, xres, cv, prm, pcol, onesb,
           out_d=None, a2pad=None, split_stats=True, defer_finish=False,
           finish_prev=None):
    """Full conv+BN+residual+maxout round.

    split_stats=True: one AllGather per output-channel block (block 0's
    mesh overlaps block 1's matmuls) — right when the CC core is warm and
    free (round 2).  split_stats=False: a single AllGather for both blocks
    — right for round 1, where the CC core ignores collectives until ~60us
    and is strictly serial, so two meshes would just queue behind the
    warmup and delay the apply."""
    sbuf, psum, sc, dram = pools
    stat = sbuf.tile([128, 2 * NCHUNK * NBLK], F32, name=f"stat{rnd}")
    if split_stats:
        s_ds = [dram.tile([128, 2 * NCHUNK], F32, name=f"sd{rnd}_{o}")
                for o in range(NBLK)]
        g_ds = [dram.tile([N_CORES, 128, 2 * NCHUNK], F32, name=f"gd{rnd}_{o}")
                for o in range(NBLK)]
    else:
        s_dm = dram.tile([128, 2 * NCHUNK * NBLK], F32, name=f"sdm{rnd}")
        g_dm = dram.tile([N_CORES, 128, 2 * NCHUNK * NBLK], F32,
                         name=f"gdm{rnd}")
    # sample-major planes: (b i h w), DoubleRow i-stride = PPIX, so each
    # matmul's rhs bounding range stays inside its own sample's planes
    t420 = apad[:, 0:2 * PLANE].rearrange(
        "p (b i n) -> p b i n", b=BPC, i=2)[:, 0, :, 0:RUN]

    def rhs_ap(off):
        return bass.AP(t420.tensor, GUARD + off, t420.ap)

    wv = wsb[:].rearrange("p (k i o) -> p k i o", k=9, i=NBLK)
    ones3 = onesb[:].rearrange("p (i o) -> p i o", i=2)

    # ---- S1 pipeline for all samples first (keeps the main MM stream
    # free of evac dependencies), then main matmuls with immediate evac ----
    s1s = {}
    for b in range(BPC):
        hs = sc.tile([128, 2 * RUN], F32, tag="hs", name="hs", bufs=2)
        for half in range(2):
            h0 = half * HHALF
            ps2 = psum.tile([128, RUN], F32, tag="ps2", name=f"ps2_{rnd}",
                            bufs=4)
            for kh in range(3):
                nc.tensor.matmul(ps2[:], ones3,
                                 rhs_ap(b * 2 * PPIX + (h0 + kh) * WPAD),
                                 start=(kh == 0), stop=(kh == 2), perf_mode=DR)
            nc.scalar.copy(hs[:, half * RUN:half * RUN + RUN], ps2[:])
        # W-direction 3-tap over the whole sample (28 rows x 30)
        hsv = hs[:].rearrange("p (h w) -> p h w", h=H)
        w3 = sc.tile([128, H * W], F32, tag="w3", name="w3", bufs=2)
        w3v = w3[:].rearrange("p (h w) -> p h w", h=H)
        nc.gpsimd.tensor_add(w3v, hsv[:, :, 0:W], hsv[:, :, 1:1 + W])
        s1 = sc.tile([128, H * W], F32, tag="s1", name="s1", bufs=BPC)
        s1v = s1[:].rearrange("p (h w) -> p h w", h=H)
        nc.vector.tensor_add(s1v, w3v, hsv[:, :, 2:2 + W])
        s1s[b] = s1

    if a2pad is not None:
        a2v = a2pad[:, GUARD:GUARD + 2 * PLANE].rearrange(
            "p (b i h w) -> p b i h w", b=BPC, i=2, h=HPAD, w=WPAD)
    else:
        a2v = None
    if out_d is not None:
        ov = out_d[:].rearrange("b c h w -> c b (h w)")
    else:
        ov = None

    # block-major main loop: block 0's AllReduce is triggered while block 1
    # is still on the PE array; both applies trail their collective.
    for oblk in range(NBLK):
        for ci in range(NCHUNK):
            b, h0 = ci // 2, (ci % 2) * HHALF
            ps = psum.tile([128, RUN], F32, tag="ps", name=f"ps{rnd}",
                           bufs=4)
            for k9 in range(9):
                kh, kw = k9 // 3, k9 % 3
                nc.tensor.matmul(
                    ps[:], wv[:, k9, :, oblk * 128:(oblk + 1) * 128],
                    rhs_ap(b * 2 * PPIX + (h0 + kh) * WPAD + kw - 1),
                    start=(k9 == 0), stop=(k9 == 8), perf_mode=DR)
            _evac(nc, sc, ps, s1s[b], stat, cv, prm, pcol, ci, oblk)
        if split_stats:
            _stats_pre(nc, rnd, oblk, stat, s_ds[oblk], g_ds[oblk])
        # the previous round's deferred maxout-finish ops slot in here:
        # after this round's block-0 collective is on its way, before the
        # block-1 stats chain — their engines are otherwise waiting
        if oblk == 0 and finish_prev is not None:
            finish_prev()
            finish_prev = None

    if split_stats:
        fins = []
        for oblk in range(NBLK):
            sss = _stats_post(nc, sbuf, rnd, oblk, g_ds[oblk], prm, pcol)
            for b in range(BPC):
                if defer_finish:
                    tp = _apply_a(nc, sc, oblk, b, cv, xres, sss, prm,
                                  pcol, a2v)
                    fins.append((oblk, b, tp, sss))
                else:
                    _apply(nc, sc, oblk, b, cv, xres, sss, prm, pcol,
                           a2v=a2v, ov=ov)
        if defer_finish:
            def finish():
                for oblk, b, tp, sss in fins:
                    _apply_b(nc, sc, oblk, b, tp, xres, sss, prm, pcol,
                             ov=ov)
            return finish
    else:
        nc.scalar.dma_start(s_dm[:], stat[:])
        nc.gpsimd.collective_compute(
            "AllGather", ALU.bypass,
            replica_groups=[list(range(N_CORES))],
            ins=[s_dm.opt()], outs=[g_dm.opt()])
        nrc = N_CORES * NCHUNK
        gst = sbuf.tile([128, NBLK * 2 * nrc], F32, name=f"gstm{rnd}")
        nc.sync.dma_start(
            gst[:].rearrange("p (o q r c) -> p o q r c", o=NBLK, q=2,
                             r=N_CORES),
            g_dm[:].rearrange("r p (o q c) -> p o q r c", o=NBLK, q=2))
        ssss = [_bn_math(nc, sbuf, rnd, o,
                         gst[:, o * 2 * nrc:o * 2 * nrc + nrc],
                         gst[:, o * 2 * nrc + nrc:(o + 1) * 2 * nrc],
                         prm, pcol)
                for o in range(NBLK)]
        # sample-major so a2 for sample b completes after two applies and
        # round 2's S1 pipeline starts immediately
        fins = []
        for b in range(BPC):
            for oblk in range(NBLK):
                if defer_finish:
                    tp = _apply_a(nc, sc, oblk, b, cv, xres, ssss[oblk],
                                  prm, pcol, a2v)
                    fins.append((oblk, b, tp, ssss[oblk]))
                else:
                    _apply(nc, sc, oblk, b, cv, xres, ssss[oblk], prm, pcol,
                           a2v=a2v, ov=ov)
        if defer_finish:
            def finish():
                for oblk, b, tp, sss in fins:
                    _apply_b(nc, sc, oblk, b, tp, xres, sss, prm, pcol,
                             ov=ov)
            return finish


def build():
    nc = bacc.Bacc("TRN2", target_bir_lowering=False, debug=False,
                   enable_asserts=True, num_devices=N_CORES)
    x_d = nc.dram_tensor("x", [BPC, C, H, W], F32, kind="ExternalInput")
    w1_d = nc.dram_tensor("w1t", [9, NBLK, 128, 256], FP8,
                          kind="ExternalInput")
    w2_d = nc.dram_tensor("w2t", [9, NBLK, 128, 256], FP8,
                          kind="ExternalInput")
    prm_d = nc.dram_tensor("prm", [128, NPRM], F32, kind="ExternalInput")
    out_d = nc.dram_tensor("out", [BPC, C, H, W], F32, kind="ExternalOutput")

    with tile.TileContext(nc) as tc:
        with (
            tc.tile_pool(name="sbuf", bufs=1) as sbuf,
            tc.tile_pool(name="psum", bufs=6, space="PSUM") as psum,
            tc.tile_pool(name="sc", bufs=2) as sc,
            tc.tile_pool(name="dram", bufs=1, space="DRAM") as dram,
        ):
            pools = (sbuf, psum, sc, dram)
            # Warmup AllGather: the CC cores ignore user collectives until
            # ~50us (NRT init) and the first mesh pays ~11us setup + cold
            # data path; this junk collective eats those costs and syncs
            # the 8 cores so the real stats meshes run warm (~5us).
            wu_i = dram.tile([1, 16], F32, name="wu_i")
            wu_o = dram.tile([N_CORES, 16], F32, name="wu_o")
            wu_s = sbuf.tile([1, 16], F32, name="wu_s")
            nc.vector.memset(wu_s[:], 0.0)
            nc.scalar.dma_start(wu_i[:], wu_s[:])
            nc.gpsimd.collective_compute(
                "AllGather", ALU.bypass,
                replica_groups=[list(range(N_CORES))],
                ins=[wu_i.opt()], outs=[wu_o.opt()])
            w1sb = sbuf.tile([128, 9 * NBLK * 256], FP8, name="w1sb")
            w2sb = sbuf.tile([128, 9 * NBLK * 256], FP8, name="w2sb")
            prm = sbuf.tile([128, NPRM], F32, name="prm")
            onesb = sbuf.tile([128, 256], FP8, name="onesb")
            nc.vector.memset(onesb[:], 1.0)
            xres = [sbuf.tile([128, BPC * PIX], F32, name=f"xres{i}")
                    for i in range(NBLK)]
            a1p = sbuf.tile([128, GUARD + 2 * PLANE + GUARD], FP8, name="a1p")
            a2p = sbuf.tile([128, GUARD + 2 * PLANE + GUARD], FP8, name="a2p")
            cv = [sbuf.tile([128, BPC * PIX], F32, name=f"cv{i}")
                  for i in range(NBLK)]

            # zero the padded activation tiles (pads must read 0); split the
            # a1 memset across DVE+GpSimd so sample-0 signing starts early
            hlen = (GUARD + 2 * PLANE + GUARD) // 2
            nc.vector.memset(a1p[:, 0:hlen].bitcast(mybir.dt.uint32), 0)
            nc.gpsimd.memset(a1p[:, hlen:].bitcast(mybir.dt.uint32), 0)
            nc.sync.dma_start(prm[:], prm_d[:])
            xv = x_d[:].rearrange("b c h w -> c b (h w)")
            for b in range(BPC):
                eng = nc.sync if b < 2 else nc.scalar
                for i in range(NBLK):
                    eng.dma_start(
                        xres[i][:, b * PIX:(b + 1) * PIX],
                        xv[i * 128:(i + 1) * 128, b])
            # w1 split across three DMA queues so no single transfer
            # gates the first matmul
            w1v = w1sb[:].rearrange("p (k i o) -> p k i o", k=9, i=NBLK)
            w1dv = w1_d[:].rearrange("k i p o -> p k i o")
            nc.scalar.dma_start(w1v[:, 0:3], w1dv[:, 0:3])
            nc.gpsimd.dma_start(w1v[:, 3:6], w1dv[:, 3:6])
            nc.sync.dma_start(w1v[:, 6:9], w1dv[:, 6:9])
            nc.gpsimd.memset(a2p[:].bitcast(mybir.dt.uint32), 0)
            nc.sync.dma_start(
                w2sb[:].rearrange("p (k i o) -> p k i o", k=9, i=NBLK),
                w2_d[:].rearrange("k i p o -> p k i o"))
            # a1 = sign(x) (+-1) into padded interior of merged fp8 tile
            a1v = a1p[:, GUARD:GUARD + 2 * PLANE].rearrange(
                "p (b i h w) -> p b i h w", b=BPC, i=2, h=HPAD, w=WPAD)
            xrvs = [xres[i][:].rearrange("p (b h w) -> p b h w", b=BPC, h=H)
                    for i in range(NBLK)]
            for b in range(BPC):
                for i in range(NBLK):
                    nc.scalar.activation(a1v[:, b, i, 1:1 + H, 1:1 + W],
                                         xrvs[i][:, b], AF.Sign)

            pcol1 = {'g': 0, 'b': 2, 'hp': 4, 'hs': 6, 'eps': 16, 'q': 20}
            pcol2 = {'g': 8, 'b': 10, 'hp': 12, 'hs': 14, 'eps': 18, 'q': 22}
            f1 = _round(nc, tc, pools, 1, a1p, w1sb, xres, cv, prm, pcol1,
                        onesb, a2pad=a2p, split_stats=False,
                        defer_finish=True)
            _round(nc, tc, pools, 2, a2p, w2sb, xres, cv, prm, pcol2, onesb,
                   out_d=out_d, split_stats=True, finish_prev=f1)

    nc.compile()
    return nc


def _prep_weight(w):
    """(O,I,3,3) fp32 -> sign lhsT (9, iblk, 128, 256) fp8 (+-1, exact),
    plus per-output-channel alpha, beta (float64)."""
    w = w.astype(np.float64)
    beta = w.mean(axis=(1, 2, 3))
    alpha = np.sqrt(((w - beta[:, None, None, None]) ** 2)
                    .mean(axis=(1, 2, 3)))
    s = np.sign(w - beta[:, None, None, None]).astype(np.float32)
    wt = s.transpose(2, 3, 1, 0).reshape(9, C, C)   # (k9, i, o)
    wt = wt.reshape(9, NBLK, 128, C)                # (k9, iblk, i, o)
    return wt.astype(ml_dtypes.float8_e4m3), alpha, beta


def make_in_maps(inputs):
    x = np.asarray(inputs['x'], np.float32)
    aa1 = float(np.asarray(inputs['alpha_a1']).reshape(-1)[0])
    aa2 = float(np.asarray(inputs['alpha_a2']).reshape(-1)[0])
    w1t, al1, be1 = _prep_weight(np.asarray(inputs['w1'], np.float32))
    w2t, al2, be2 = _prep_weight(np.asarray(inputs['w2'], np.float32))
    prm = np.zeros((128, NPRM), np.float32)
    f1 = 1.0 / (aa1 * al1)      # z scale relative to the true conv output
    f2 = 1.0 / (aa2 * al2)
    p1 = np.asarray(inputs['pos1'], np.float64)
    n1 = np.asarray(inputs['neg1'], np.float64)
    p2 = np.asarray(inputs['pos2'], np.float64)
    n2 = np.asarray(inputs['neg2'], np.float64)
    cols = ((0, np.asarray(inputs['g1'], np.float64)),
            (2, np.asarray(inputs['b1'], np.float64)),
            (4, (p1 - n1) / 2),
            (6, (p1 + n1) / 2),
            (8, np.asarray(inputs['g2'], np.float64)),
            (10, np.asarray(inputs['b2'], np.float64)),
            (12, (p2 - n2) / 2),
            (14, (p2 + n2) / 2),
            (16, BN_EPS * f1 * f1),
            (18, BN_EPS * f2 * f2),
            (20, be1 / al1),
            (22, be2 / al2))
    for base, arr in cols:
        prm[:, base] = arr[:128]
        prm[:, base + 1] = arr[128:]
    in_maps = []
    for c in range(N_CORES):
        in_maps.append({
            'x': np.ascontiguousarray(x[c * BPC:(c + 1) * BPC]),
            'w1t': w1t, 'w2t': w2t, 'prm': prm,
        })
    return in_maps


_CACHE = {}


def kernel(**inputs):
    in_maps = make_in_maps(inputs)
    if 'run' not in _CACHE:
        nc = build()
        _CACHE['nc'] = nc
        _CACHE['run'] = _make_runner(nc)
    outs = _CACHE['run'](in_maps)
    return np.concatenate([outs[c] for c in range(N_CORES)], axis=0)


def _make_runner(nc):
    """Build a cached PJRT executable (same path run_bass_kernel_spmd takes
    under axon, via bass2jax) so repeat calls don't re-trace."""
    import jax
    import jax.numpy as jnp
    from jax.sharding import Mesh, PartitionSpec
    from jax.experimental.shard_map import shard_map
    from concourse import bass2jax

    bass2jax.install_neuronx_cc_hook()
    partition_name = (nc.partition_id_tensor.name
                      if nc.partition_id_tensor else None)
    in_names = []
    out_names = []
    out_avals = []
    for alloc in nc.m.functions[0].allocations:
        if not isinstance(alloc, mybir.MemoryLocationSet):
            continue
        name = alloc.memorylocations[0].name
        if alloc.kind == "ExternalInput":
            if name != partition_name:
                in_names.append(name)
        elif alloc.kind == "ExternalOutput":
            shape = tuple(alloc.tensor_shape)
            dtype = mybir.dt.np(alloc.dtype)
            out_names.append(name)
            out_avals.append(jax.core.ShapedArray(shape, dtype))
    n_params = len(in_names)
    all_names = in_names + out_names
    if partition_name is not None:
        all_names = all_names + [partition_name]

    def _body(*args):
        operands = list(args)
        if partition_name is not None:
            operands.append(bass2jax.partition_id_tensor())
        outs = bass2jax._bass_exec_p.bind(
            *operands,
            out_avals=tuple(out_avals),
            in_names=tuple(all_names),
            out_names=tuple(out_names),
            lowering_input_output_aliases=(),
            sim_require_finite=True,
            sim_require_nnan=True,
            nc=nc,
        )
        return tuple(outs)

    devices = jax.devices()[:N_CORES]
    mesh = Mesh(np.asarray(devices), ("core",))
    n_outs = len(out_names)
    sharded = jax.jit(
        shard_map(_body, mesh=mesh,
                  in_specs=(PartitionSpec("core"),) * (n_params + n_outs),
                  out_specs=(PartitionSpec("core"),) * n_outs,
                  check_rep=False),
        donate_argnums=tuple(range(n_params, n_params + n_outs)),
        keep_unused=True,
    )
    sharded_nodonate = jax.jit(
        shard_map(_body, mesh=mesh,
                  in_specs=(PartitionSpec("core"),) * (n_params + n_outs),
                  out_specs=(PartitionSpec("core"),) * n_outs,
                  check_rep=False),
        keep_unused=True,
    )

    def run(in_maps):
        concat_in = [
            np.concatenate([np.asarray(in_maps[c][n]) for c in range(N_CORES)],
                           axis=0)
            for n in in_names
        ]
        concat_zeros = [
            np.zeros((N_CORES * a.shape[0], *a.shape[1:]), a.dtype)
            for a in out_avals
        ]
        out_arrs = sharded(*concat_in, *concat_zeros)
        i = out_names.index("out")
        full = np.asarray(out_arrs[i]).reshape(N_CORES, *out_avals[i].shape)
        return [full[c] for c in range(N_CORES)]

    def stage(in_maps):
        """device_put inputs once; return a dispatch closure for timing."""
        from jax.sharding import NamedSharding
        sh = NamedSharding(mesh, PartitionSpec("core"))
        concat_in = [
            jax.device_put(np.concatenate(
                [np.asarray(in_maps[c][n]) for c in range(N_CORES)], axis=0), sh)
            for n in in_names
        ]
        concat_zeros = [
            jax.device_put(
                np.zeros((N_CORES * a.shape[0], *a.shape[1:]), a.dtype), sh)
            for a in out_avals
        ]

        def dispatch():
            return sharded_nodonate(*concat_in, *concat_zeros)

        return dispatch

    run.stage = stage
    return run
